# revision 9
# baseline (speedup 1.0000x reference)
"""GAT message-passing kernel for TRN2 (8 NeuronCores, SPMD).

Algorithm (matches the jax reference up to a softmax shift, which cancels):
  proj = src @ W_src.T ; s_src[n,h] = proj[n,h,:].a_src[h] ; s_trg[n,h] = trg[n].w_t[h]
  score_e = leakyrelu(s_src[si]+s_trg[ti]) ; p_e = exp(score_e - C_OFF)
  out[t,h,:] = sum_{e: ti=t} p_e * proj[si_e,h,:] / (sum p_e + eps)

Sharding: edges sorted by target; core c owns targets [c*TPC,(c+1)*TPC).
P0 builds a bf16 proj table [NPAD,128] in DRAM with node ids remapped to
partition-major order (n -> (n%128)*NT0 + n//128) so table writes are large
contiguous per-partition descriptors. P1 runs per superwindow (SWW target
windows): one dma_gather call per (sw, slab) pulls all needed 256B proj rows;
s_src is recomputed on-chip (mult+reduce vs a_src); s_trg expands per edge via
a one-hot matmul whose operand is built on-chip (tl stream + partition
broadcast); a one-hot matmul segment-sums numerator||denominator per window.
"""
import os
import numpy as np
import ml_dtypes

import concourse.bacc as bacc
import concourse.mybir as mybir
import concourse.tile as tile
from concourse.bass_utils import run_bass_kernel_spmd

BF16 = mybir.dt.bfloat16
F32 = mybir.dt.float32
I16 = mybir.dt.int16

NH, FOUT, D = 8, 16, 128
HF = NH * FOUT  # 128
NEG_SLOPE = 0.2
C_OFF = 16.0
SLAB = 32768
NQ = 4             # SWDGE queues
SWW = 3            # target windows per superwindow

LAST_EXEC_NS = None


def _install_trace_shim():
    """Register the axon NTFF profile hook (missing antenv.axon_hooks shim)."""
    import sys
    import types

    if "antenv.axon_hooks" in sys.modules:
        return True
    try:
        mod = types.ModuleType("antenv.axon_hooks")
        mod._hook = None
        mod.set_axon_ntff_profile_hook = lambda h: setattr(mod, "_hook", h)
        mod.get_axon_ntff_profile_hook = lambda: mod._hook
        from trn_agent_boot.trn_boot import _ntff_profile_via_ctypes

        mod._hook = _ntff_profile_via_ctypes("/opt/axon/libaxon_pjrt.so")
        sys.modules["antenv.axon_hooks"] = mod
        import concourse.bass_utils as bu

        bu.upload_artifacts = lambda tmpdir: tmpdir
        return True
    except Exception:
        return False


def _wrap_idx(v):
    """[ct*128] int array -> [128, ct*8] int16 wrapped+replicated layout."""
    w = np.asarray(v, dtype=np.int16).reshape(-1, 16).T  # [16, ct*8]
    return np.tile(w, (8, 1))


def build_schedule(si, ti, N, ncores):
    """SPMD schedule: slot layout is (sw, slab, window)-major so one gather
    call per (sw, slab) covers a contiguous destination range."""
    TPC = N // ncores
    WPC = (TPC + 127) // 128
    NPAD = ((N + 127) // 128) * 128
    NT0 = NPAD // 128
    nslabs = (NPAD + SLAB - 1) // SLAB
    NSW = (WPC + SWW - 1) // SWW

    si = np.asarray(si, dtype=np.int64)
    ti = np.asarray(ti, dtype=np.int64)
    # remap source node id to partition-major table order
    sir = (si % 128) * NT0 + si // 128
    core = ti // TPC
    tloc = ti - core * TPC
    w_of = tloc >> 7
    s_of = sir >> 15

    counts = np.zeros((ncores, WPC, nslabs), dtype=np.int64)
    np.add.at(counts, (core, w_of, s_of), 1)
    NT = np.ceil(counts.max(axis=0) / 128).astype(np.int64)  # [WPC, nslabs]
    NW = NT.sum(axis=1)

    # slot order: sw-major, then slab, then window-in-sw
    slot0 = np.zeros((WPC, nslabs), dtype=np.int64)  # first tile slot of (w,s)
    g1calls = []   # per sw: list of (s, k0, ct) with k0 global tile slot
    sw_tiles = []  # per sw: (tile0, Tsw)
    k = 0
    for sw in range(NSW):
        wlo, whi = sw * SWW, min((sw + 1) * SWW, WPC)
        t0 = k
        calls = []
        for s in range(nslabs):
            k0 = k
            for w in range(wlo, whi):
                slot0[w, s] = k
                k += int(NT[w, s])
            if k > k0:
                calls.append((s, k0, k - k0))
        g1calls.append(calls)
        sw_tiles.append((t0, k - t0))
    T_total = k
    TSWMAX = max(t for _, t in sw_tiles)
    SECMAX = max(ct for c in g1calls for (_, _, ct) in c)

    # per-window MM runs: list of (slot, cnt, window) per slab
    runs = []
    for w in range(WPC):
        r = [(int(slot0[w, s]), int(NT[w, s])) for s in range(nslabs)
             if NT[w, s] > 0]
        runs.append(r)
    # per-slot window id (for pse rhs selection)
    slotw = np.zeros(T_total, dtype=np.int64)
    for w in range(WPC):
        for (k0, ct) in runs[w]:
            slotw[k0 : k0 + ct] = w

    per_core = []
    for c in range(ncores):
        m = core == c
        csi = sir[m]
        cw, cs = w_of[m], s_of[m]
        ctl = tloc[m] & 127
        order = np.lexsort((cs, cw))
        csi, cw, cs, ctl = (a[order] for a in (csi, cw, cs, ctl))

        si_loc = np.zeros(T_total * 128, dtype=np.int64)
        tl = np.full(T_total * 128, 255, dtype=np.int64)

        keys = cw * nslabs + cs
        starts = np.searchsorted(keys, np.arange(WPC * nslabs))
        ends = np.searchsorted(keys, np.arange(WPC * nslabs), side="right")
        for w in range(WPC):
            for s in range(nslabs):
                a, b = starts[w * nslabs + s], ends[w * nslabs + s]
                cnt = b - a
                if cnt:
                    p0 = int(slot0[w, s]) * 128
                    si_loc[p0 : p0 + cnt] = csi[a:b] - cs[a:b] * SLAB
                    tl[p0 : p0 + cnt] = ctl[a:b]

        blocks = []
        for sw in range(NSW):
            for (s, k0, ct) in g1calls[sw]:
                blocks.append(_wrap_idx(si_loc[k0 * 128 : (k0 + ct) * 128]))
        idx1 = np.concatenate(blocks, axis=1) if blocks else np.zeros((128, 8), np.int16)
        tl_bf = tl.reshape(T_total, 128).T.astype(np.float32).astype(ml_dtypes.bfloat16)
        tlf = tl.astype(np.float32).astype(ml_dtypes.bfloat16).reshape(1, -1)
        per_core.append((idx1, tl_bf, tlf))

    return dict(TPC=TPC, WPC=WPC, NPAD=NPAD, NT0=NT0, nslabs=nslabs, NSW=NSW,
                NT=NT, NW=NW, T_total=T_total, TSWMAX=TSWMAX, SECMAX=SECMAX,
                g1calls=g1calls, sw_tiles=sw_tiles, runs=runs, slotw=slotw), per_core


def build_nc(sched):
    WPC, NPAD, NT0 = sched["WPC"], sched["NPAD"], sched["NT0"]
    NSW, T_total, TSWMAX = sched["NSW"], sched["T_total"], sched["TSWMAX"]
    SECMAX = sched["SECMAX"]
    g1calls, sw_tiles, runs, NW, slotw = (sched["g1calls"], sched["sw_tiles"],
                                          sched["runs"], sched["NW"],
                                          sched["slotw"])
    LROWS = WPC * 128

    nc = bacc.Bacc("TRN2", target_bir_lowering=False, num_swdge_queues=NQ)
    srcT = nc.declare_dram_parameter("srcT", [128, NPAD], BF16, isOutput=False)
    trgTl = nc.declare_dram_parameter("trgTl", [128, LROWS], BF16, isOutput=False)
    wext = nc.declare_dram_parameter("wext", [128, 144], BF16, isOutput=False)
    iota = nc.declare_dram_parameter("iota", [128, 128], BF16, isOutput=False)
    iotac = nc.declare_dram_parameter("iotac", [128, 1], F32, isOutput=False)
    idx1 = nc.declare_dram_parameter("idx1", [128, max(T_total * 8, 8)], I16, isOutput=False)
    tlp = nc.declare_dram_parameter("tl", [128, max(T_total, 1)], BF16, isOutput=False)
    tlf = nc.declare_dram_parameter("tlf", [1, max(T_total * 128, 128)], BF16, isOutput=False)
    outp = nc.declare_dram_parameter("out", [128, WPC * HF], F32, isOutput=True)
    table = nc.dram_tensor("table", [NPAD, 256], BF16)
    table_v = table[:, :].rearrange("(p k) c -> p k c", p=128)  # [128, NT0, 128]
    outp_v = outp[:, :].rearrange("p (w c) -> p w c", c=HF)

    qrr = [0]

    def next_q():
        q = qrr[0]
        qrr[0] = (q + 1) % NQ
        return q

    with tile.TileContext(nc) as tc:
        with tc.tile_pool(name="const", bufs=1) as cp:
            wext_sb = cp.tile([128, 144], BF16)
            nc.sync.dma_start(out=wext_sb[:], in_=wext[:, :])
            iota_sb = cp.tile([128, 128], BF16)
            nc.sync.dma_start(out=iota_sb[:], in_=iota[:, :])
            iotac_sb = cp.tile([128, 1], F32)
            nc.sync.dma_start(out=iotac_sb[:], in_=iotac[:, :])
            tl_sb = cp.tile([128, max(T_total, 1)], BF16)
            nc.sync.dma_start(out=tl_sb[:], in_=tlp[:, :])
            cbias = cp.tile([128, 1], F32)
            nc.vector.memset(cbias[:], -C_OFF)
            ones_sb = cp.tile([128, 128], BF16)
            nc.vector.memset(ones_sb[:], 1.0)
            strg_sb = cp.tile([128, WPC, 16], BF16)

            # ---- P0b: per-core s_trg (hi|lo bf16) kept in SBUF ----
            with (
                tc.tile_pool(name="pbin", bufs=3) as pbin,
                tc.tile_pool(name="pbps", bufs=3, space="PSUM") as pbps,
                tc.tile_pool(name="pbl", bufs=3) as pbl,
            ):
                GB = 3
                j = 0
                while j < WPC:
                    g = min(GB, WPC - j)
                    t_t = pbin.tile([128, 128 * g], BF16, tag="t", name=f"t{j}")
                    nc.sync.dma_start(out=t_t[:, :], in_=trgTl[:, j * 128 : (j + g) * 128])
                    psB = pbps.tile([128, 8 * g], F32, tag="psB", name=f"pb{j}")
                    for u in range(g):
                        nc.tensor.matmul(
                            out=psB[:, u * 8 : (u + 1) * 8],
                            lhsT=t_t[:, u * 128 : (u + 1) * 128],
                            rhs=wext_sb[:, 136:144],
                            start=True, stop=True,
                        )
                    psB_r = psB[:].rearrange("p (k c) -> p k c", c=8)
                    nc.vector.tensor_copy(out=strg_sb[:, j : j + g, 0:8], in_=psB_r)
                    lob = pbl.tile([128, g, 8], F32, tag="lob", name=f"lo{j}")
                    nc.vector.tensor_tensor(
                        out=lob[:], in0=psB_r, in1=strg_sb[:, j : j + g, 0:8],
                        op=mybir.AluOpType.subtract,
                    )
                    nc.scalar.copy(out=strg_sb[:, j : j + g, 8:16], in_=lob[:])
                    j += g

            # ---- P0: bf16 proj table [NPAD,128], partition-major node order ----
            with (
                tc.tile_pool(name="p0in", bufs=3) as p0in,
                tc.tile_pool(name="p0row", bufs=3) as p0row,
                tc.tile_pool(name="p0ps", bufs=3, space="PSUM") as p0ps,
            ):
                G0 = 3
                j = 0
                gi = 0
                row = None
                jp = 0
                half = 0
                while j < NT0:
                    g = min(G0, NT0 - j)
                    s_t = p0in.tile([128, 128 * g], BF16, tag="s", name=f"s{j}")
                    nc.sync.dma_start(out=s_t[:, :], in_=srcT[:, j * 128 : (j + g) * 128])
                    psA = p0ps.tile([128, g, 136], F32, tag="psA", name=f"pa{j}")
                    for u in range(g):
                        nc.tensor.matmul(
                            out=psA[:, u, :],
                            lhsT=s_t[:, u * 128 : (u + 1) * 128],
                            rhs=wext_sb[:, 0:136],
                            start=True, stop=True,
                        )
                    if gi % 2 == 0:
                        row = p0row.tile([128, 2 * G0, 256], BF16, tag="row", name=f"r{j}")
                        jp, half = j, 0
                    else:
                        half = G0
                    nc.scalar.copy(out=row[:, half : half + g, 0:128], in_=psA[:, :, 0:128])
                    row_f32 = row[:].bitcast(F32)
                    nc.scalar.copy(out=row_f32[:, half : half + g, 64:72], in_=psA[:, :, 128:136])
                    if gi % 2 == 1 or j + g >= NT0:
                        nwrite = half + g
                        nc.scalar.dma_start(
                            out=table_v[:, jp : jp + nwrite, :], in_=row[:, 0:nwrite, :]
                        )
                    gi += 1
                    j += g

            # ---- P1: edge pass, one superwindow (SWW windows) at a time ----
            with (
                tc.tile_pool(name="g1p", bufs=2) as g1p,
                tc.tile_pool(name="idxp", bufs=2) as idxp,
                tc.tile_pool(name="ohtp", bufs=2) as ohtp,
                tc.tile_pool(name="ohp", bufs=2) as ohp,
                tc.tile_pool(name="ssp", bufs=2) as ssp,
                tc.tile_pool(name="e2p", bufs=2) as e2p,
                tc.tile_pool(name="whp", bufs=2) as whp,
                tc.tile_pool(name="psep", bufs=2, space="PSUM") as psep,
                tc.tile_pool(name="bcp", bufs=2, space="PSUM") as bcp,
                tc.tile_pool(name="pswp", bufs=2, space="PSUM") as pswp,
                tc.tile_pool(name="epi", bufs=2) as epi,
            ):
                c1off = 0
                for sw in range(NSW):
                    t0, tsw = sw_tiles[sw]
                    wlo, whi = sw * SWW, min((sw + 1) * SWW, WPC)
                    nws = whi - wlo
                    outt = epi.tile([128, SWW, HF], F32, tag="outt", name=f"o{sw}")

                    # per-sw gather index slice from DRAM
                    idxt = idxp.tile([128, max(TSWMAX * 8, 8)], I16, tag="ix", name=f"ix{sw}")
                    nc.sync.dma_start(
                        out=idxt[:, 0 : tsw * 8],
                        in_=idx1[:, c1off : c1off + tsw * 8],
                    )

                    G1 = g1p.tile([128, TSWMAX, 256], BF16, tag="g1", name=f"g1_{sw}")
                    G1f = G1[:].bitcast(F32)
                    for (s, k0, ct) in g1calls[sw]:
                        sb = s * SLAB
                        se = min(sb + SLAB, NPAD)
                        nc.gpsimd.dma_gather(
                            G1[:, k0 - t0 : k0 - t0 + ct, :],
                            table[sb:se, :],
                            idxt[:, (k0 - t0) * 8 : (k0 - t0 + ct) * 8],
                            ct * 128, ct * 128, 256,
                            single_packet=False,
                            queue_num=next_q(),
                        )
                    c1off += tsw * 8

                    # ohT[t, e] = (tl[e] == t): stage flat tl on partition 0
                    # of ohT, replicate to all partitions via a K=1 ones
                    # matmul into PSUM, compare from PSUM vs per-partition
                    # iota. 4-tile chunks keep each PSUM tile within a bank.
                    ohT = ohtp.tile([128, TSWMAX * 128], BF16, tag="ohT", name=f"ohT{sw}")
                    oh = ohp.tile([128, TSWMAX, 128], BF16, tag="oh", name=f"oh{sw}")
                    nc.sync.dma_start(
                        out=ohT[0:1, 0 : tsw * 128],
                        in_=tlf[0:1, t0 * 128 : (t0 + tsw) * 128],
                    )
                    iota_b = iota_sb[:].rearrange("p (o c) -> p o c", o=1).to_broadcast(
                        [128, tsw, 128]
                    )
                    tl_b = tl_sb[:, t0 : t0 + tsw].rearrange(
                        "p (k o) -> p k o", o=1
                    ).to_broadcast([128, tsw, 128])
                    nc.vector.tensor_tensor(
                        out=oh[:, 0:tsw, :], in0=iota_b, in1=tl_b,
                        op=mybir.AluOpType.is_equal,
                    )
                    c0 = 0
                    while c0 < tsw * 128:
                        cols = min(512, tsw * 128 - c0)
                        bc = bcp.tile([128, 512], F32, tag="bc", name=f"bc{sw}_{c0}")
                        nc.tensor.matmul(
                            out=bc[:, 0:cols],
                            lhsT=ones_sb[0:1, 0:128],
                            rhs=ohT[0:1, c0 : c0 + cols],
                            start=True, stop=True,
                        )
                        nc.vector.tensor_scalar(
                            out=ohT[:, c0 : c0 + cols], in0=bc[:, 0:cols],
                            scalar1=iotac_sb[:, 0:1], scalar2=None,
                            op0=mybir.AluOpType.is_equal,
                        )
                        c0 += cols

                    wt = whp.tile([128, TSWMAX, 136], BF16, tag="wt", name=f"wt{sw}")

                    # s_trg per edge (one-hot matmul), then scores, per slab
                    # section (slots are contiguous per section).
                    sc = ssp.tile([128, TSWMAX, 8], F32, tag="sc", name=f"sc{sw}")
                    for (s, k0, ct) in g1calls[sw]:
                        pse = psep.tile([128, SECMAX * 16], F32, tag="pse",
                                        name=f"pse{sw}_{s}")
                        for u in range(ct):
                            kk = k0 - t0 + u
                            w = int(slotw[k0 + u])
                            nc.tensor.matmul(
                                out=pse[:, u * 16 : (u + 1) * 16],
                                lhsT=ohT[:, kk * 128 : (kk + 1) * 128],
                                rhs=strg_sb[:, w, :], start=True, stop=True,
                            )
                        pse_r = pse[:].rearrange("p (k c) -> p k c", c=16)
                        nc.vector.tensor_tensor(
                            out=sc[:, k0 - t0 : k0 - t0 + ct, :],
                            in0=G1f[:, k0 - t0 : k0 - t0 + ct, 64:72],
                            in1=pse_r[:, 0:ct, 0:8], op=mybir.AluOpType.add,
                        )
                        nc.vector.tensor_tensor(
                            out=sc[:, k0 - t0 : k0 - t0 + ct, :],
                            in0=sc[:, k0 - t0 : k0 - t0 + ct, :],
                            in1=pse_r[:, 0:ct, 8:16], op=mybir.AluOpType.add,
                        )

                    # exp(leakyrelu(s)-C) = max(exp(s-C), exp(.2s-C))
                    e1 = ssp.tile([128, TSWMAX, 8], F32, tag="e1", name=f"e1_{sw}")
                    nc.scalar.activation(
                        e1[:, 0:tsw, :], sc[:, 0:tsw, :],
                        mybir.ActivationFunctionType.Exp, bias=cbias[:, 0:1],
                    )
                    e2 = e2p.tile([128, TSWMAX, 8], F32, tag="e2", name=f"e2_{sw}")
                    nc.scalar.activation(
                        e2[:, 0:tsw, :], sc[:, 0:tsw, :],
                        mybir.ActivationFunctionType.Exp, bias=cbias[:, 0:1],
                        scale=NEG_SLOPE,
                    )
                    nc.vector.tensor_tensor(
                        out=wt[:, 0:tsw, 128:136], in0=e1[:, 0:tsw, :],
                        in1=e2[:, 0:tsw, :], op=mybir.AluOpType.max,
                    )
                    e_b = wt[:, 0:tsw, 128:136].rearrange(
                        "p k (h o) -> p k h o", o=1
                    ).to_broadcast([128, tsw, 8, 16])
                    nc.vector.tensor_tensor(
                        out=wt[:, 0:tsw, 0:128].rearrange("p k (h f) -> p k h f", f=16),
                        in0=G1[:, 0:tsw, 0:128].rearrange("p k (h f) -> p k h f", f=16),
                        in1=e_b, op=mybir.AluOpType.mult,
                    )

                    # segment-sum per window + epilogue
                    for w in range(wlo, whi):
                        wi = w - wlo
                        if int(NW[w]) == 0:
                            nc.vector.memset(outt[:, wi, :], 0.0)
                            continue
                        psw = pswp.tile([128, 136], F32, tag="ps", name=f"ps{w}")
                        tot = int(NW[w])
                        done = 0
                        for (k0, ct) in runs[w]:
                            for u in range(ct):
                                kk = k0 - t0 + u
                                nc.tensor.matmul(
                                    out=psw[:], lhsT=oh[:, kk, :], rhs=wt[:, kk, :],
                                    start=(done == 0), stop=(done == tot - 1),
                                )
                                done += 1
                        dn = epi.tile([128, 8], F32, tag="dn", name=f"dn{w}")
                        nc.vector.tensor_scalar_add(out=dn[:], in0=psw[:, 128:136], scalar1=1e-16)
                        rc = epi.tile([128, 8], F32, tag="rc", name=f"rc{w}")
                        nc.vector.reciprocal(out=rc[:], in_=dn[:])
                        rc_b = rc[:].rearrange("p (h o) -> p h o", o=1).to_broadcast([128, 8, 16])
                        nc.vector.tensor_tensor(
                            out=outt[:, wi, :].rearrange("p (h f) -> p h f", f=16),
                            in0=psw[:, 0:HF].rearrange("p (h f) -> p h f", f=16),
                            in1=rc_b, op=mybir.AluOpType.mult,
                        )
                    nc.sync.dma_start(out=outp_v[:, wlo:whi, :], in_=outt[:, 0:nws, :])
    nc.compile()
    return nc


def host_prep(trg, src, W_trg, W_src, a_src, a_trg, N, ncores, TPC, WPC):
    NPAD = ((N + 127) // 128) * 128
    LROWS = WPC * 128
    src2 = np.asarray(src, dtype=np.float32).reshape(-1, D)[:N]
    trg2 = np.asarray(trg, dtype=np.float32).reshape(-1, D)[:N]
    W_src = np.asarray(W_src, dtype=np.float32)
    W_trg = np.asarray(W_trg, dtype=np.float32)
    a_src = np.asarray(a_src, dtype=np.float32)
    a_trg = np.asarray(a_trg, dtype=np.float32)
    w_s = np.einsum("hf,hfd->hd", a_src, W_src.reshape(NH, FOUT, D))
    w_t = np.einsum("hf,hfd->hd", a_trg, W_trg.reshape(NH, FOUT, D))
    wext = np.zeros((128, 144), dtype=np.float32)
    wext[:, 0:HF] = W_src.T
    wext[:, 128:136] = w_s.T
    wext[:, 136:144] = w_t.T
    bf = ml_dtypes.bfloat16
    srcT = np.zeros((128, NPAD), dtype=np.float32)
    srcT[:, :N] = src2.T
    trgTls = []
    for c in range(ncores):
        t = np.zeros((128, LROWS), dtype=np.float32)
        t[:, :TPC] = trg2[c * TPC : (c + 1) * TPC].T
        trgTls.append(t.astype(bf))
    iota = np.tile(np.arange(128, dtype=np.float32), (128, 1))
    return (srcT.astype(bf), trgTls, wext.astype(bf), iota.astype(bf))


_CACHE = {}


def run_graph(trg, src, edge_index, W_trg, W_src, a_src, a_trg, N, ncores,
              trace=False):
    global LAST_EXEC_NS
    si = np.asarray(edge_index[0], dtype=np.int64)
    ti = np.asarray(edge_index[1], dtype=np.int64)
    sched, per_core = build_schedule(si, ti, N, ncores)
    TPC, WPC, T_total = sched["TPC"], sched["WPC"], sched["T_total"]

    srcT, trgTls, wext, iota = host_prep(
        trg, src, W_trg, W_src, a_src, a_trg, N, ncores, TPC, WPC
    )

    key = (N, ncores, T_total, tuple(int(x) for x in sched["NW"]),
           tuple((s, k0, ct) for c in sched["g1calls"] for (s, k0, ct) in c))
    if key not in _CACHE:
        _CACHE[key] = build_nc(sched)
    nc = _CACHE[key]

    in_maps = []
    for c in range(ncores):
        idx1, tl_bf, tlf = per_core[c]
        i1 = np.zeros((128, max(T_total * 8, 8)), dtype=np.int16)
        i1[:, : idx1.shape[1]] = idx1
        tlz = np.full((128, max(T_total, 1)), 255.0, dtype=ml_dtypes.bfloat16)
        tlz[:, : tl_bf.shape[1]] = tl_bf
        tfz = np.full((1, max(T_total * 128, 128)), 255.0, dtype=ml_dtypes.bfloat16)
        tfz[:, : tlf.shape[1]] = tlf
        in_maps.append(
            {"srcT": srcT, "trgTl": trgTls[c], "wext": wext,
             "iota": iota, "idx1": i1, "tl": tlz, "tlf": tfz,
             "iotac": np.arange(128, dtype=np.float32).reshape(128, 1)}
        )

    if trace:
        trace = _install_trace_shim()
    res = run_bass_kernel_spmd(nc, in_maps, core_ids=list(range(ncores)), trace=trace)
    LAST_EXEC_NS = res.exec_time_ns
    out = np.zeros((N, HF), dtype=np.float32)
    for c in range(ncores):
        o = res.results[c]["out"].reshape(128, WPC, HF)
        full = np.transpose(o, (1, 0, 2)).reshape(WPC * 128, HF)
        out[c * TPC : (c + 1) * TPC] = full[:TPC]
    return out


def kernel(trg, src, edge_index, W_trg, W_src, a_src, a_trg):
    N = 100000
    out = run_graph(trg, src, edge_index, W_trg, W_src, a_src, a_trg, N, 8,
                    trace=bool(os.environ.get("KERNEL_TRACE")))
    return out.reshape(1, N, HF)
